# revision 2
# baseline (speedup 1.0000x reference)
"""DDiT block kernel v2 for 8 Trainium2 NeuronCores.

Same sharding as v1: data-parallel over (batch, sequence-half). Key changes:
- rope half-swap done with a PE permutation matmul (no SBUF->SBUF DMAs)
- attention outputs land in the right partitions via identity-matmul shift
- biases/cos/sin fused into single DMAs; stores + psum->sbuf copies moved to
  the Pool (gpsimd) queue; some ACT work moved to DVE to balance engines.
"""

import numpy as np
import ml_dtypes

BF = ml_dtypes.bfloat16

B, S, D, H, HD = 4, 1024, 1024, 16, 64
Q = 512
KO = 8
MLP = 4096
LN_EPS = 1e-5

_CACHE = {}


def _pieces(W, m_piece):
    K, M = W.shape
    ko = K // 128
    Wr = np.asarray(W, np.float32).reshape(ko, 128, M).transpose(1, 0, 2)
    n = M // m_piece
    out = Wr.reshape(128, ko, n, m_piece).transpose(2, 0, 1, 3)
    return np.ascontiguousarray(out.astype(BF))


def _pvec(v):
    v = np.asarray(v, np.float32)
    return np.ascontiguousarray(v.reshape(-1, 128).T)


def _perm_mats():
    """[2,128,128] bf16: [0]=rope half-swap permutation, [1]=64x64 identity."""
    P = np.zeros((128, 128), np.float32)
    for j in range(128):
        blk, r = j // 64, j % 64
        k = blk * 64 + (r + 32 if r < 32 else r - 32)
        P[k, j] = 1.0
    I = np.zeros((128, 128), np.float32)
    I[:64, :64] = np.eye(64)
    return np.ascontiguousarray(np.stack([P, I]).astype(BF))


def _build_program(repeat=1):
    import concourse.bass as bass
    import concourse.mybir as mybir
    import concourse.tile as tile
    from concourse import bacc

    f32 = mybir.dt.float32
    bf = mybir.dt.bfloat16
    AF = mybir.ActivationFunctionType
    ALU = mybir.AluOpType
    ts = bass.ts

    nc = bacc.Bacc("TRN2", target_bir_lowering=False, debug=False,
                   enable_asserts=False)

    def din(name, shape, dt=bf):
        return nc.dram_tensor(name, shape, dt, kind="ExternalInput").ap()

    xb_d = din("xb", [D, S])                      # bf16 x, feature-major
    xs_d = din("xskip", [D, Q], f32)              # f32 skip, feature-major
    wq_d = din("wq", [2, 128, KO, 512])
    wk_d = din("wk", [2, 128, KO, 512])
    wv_d = din("wv", [2, 128, KO, 512])
    wo_d = din("wao", [2, 128, KO, 512])
    w1_d = din("wm1", [8, 128, KO, 512])
    w2_d = din("wm2", [8, 128, 32, 128])
    cs_d = din("cs", [128, 2 * S])                # cos | sin, bf16
    bias_d = din("bias", [128, 80], f32)          # bq|bk|bao|gmsa|bm1|bm2|gmlp
    pm_d = din("pm", [2, 128, 128])               # swap perm | id64
    yt_d = nc.dram_tensor("yt", [D, Q], f32, kind="ExternalOutput").ap()

    with tile.TileContext(nc) as tc:
        with tc.tile_pool(name="sb", bufs=1) as sb, \
             tc.tile_pool(name="ps", bufs=1, space="PSUM") as ps:
            for _rep in range(repeat):

                def psum():
                    return ps.tile([128, 512], f32, tag="p", bufs=4, name="pt")

                def psum2():
                    return ps.tile([128, 1024], f32, tag="p2", bufs=2, name="pt2")

                def tmpf():
                    return sb.tile([128, 512], f32, tag="tmpf", bufs=4, name="tf")

                def rep2(ap):
                    import bass_rust
                    a = ap.copy()
                    p = [tuple(x) for x in a.ap]
                    a.ap = bass_rust.VecI64Pair([p[0], (0, 2), p[1]])
                    return a

                # ---- P0: input DMAs (xb chunks feed LN1 asap) ----
                xb8 = []
                for ko in range(KO):
                    t = sb.tile([128, S], bf, tag="xb8", bufs=8, name="xb")
                    nc.sync.dma_start(
                        t[:], xb_d.rearrange("(ko p) t -> p ko t", p=128)[:, ko, :])
                    xb8.append(t)
                xskip = sb.tile([128, KO, Q], f32, tag="xskip", bufs=1)
                nc.scalar.dma_start(xskip[:], xs_d.rearrange("(ko p) t -> p ko t", p=128))
                ones_b = sb.tile([128, 128], bf, tag="ones", bufs=2)
                nc.vector.memset(ones_b[:], 1.0)
                eps_ap = sb.tile([128, 1], f32, tag="eps", bufs=1)
                nc.vector.memset(eps_ap[:], LN_EPS)
                cssb = sb.tile([128, 2 * S], bf, tag="cs", bufs=1)
                nc.scalar.dma_start(cssb[:], cs_d[:])
                csb = cssb[:, 0:S]
                ssb = cssb[:, S:2 * S]
                bias_s = sb.tile([128, 80], f32, tag="bias", bufs=1, name="bias")
                nc.scalar.dma_start(bias_s[:], bias_d[:])
                bq_s = bias_s[:, 0:8]
                bk_s = bias_s[:, 8:16]
                bo_s = bias_s[:, 16:24]
                gm_s = bias_s[:, 24:32]
                b1_s = bias_s[:, 32:64]
                b2_s = bias_s[:, 64:72]
                gp_s = bias_s[:, 72:80]
                pm_sb = sb.tile([128, 2, 128], bf, tag="perm", bufs=1, name="pm")
                nc.scalar.dma_start(pm_sb[:], pm_d.rearrange("k p j -> p k j"))
                pswap = pm_sb[:, 0, :]
                id64 = pm_sb[0:64, 1, 0:64]

                def wpiece(dram, i, shape, tag="w8", bufs=6, eng=nc.sync):
                    t = sb.tile(shape, bf, tag=tag, bufs=bufs, name="w")
                    eng.dma_start(t[:], dram[i])
                    return t

                wq_sb = [wpiece(wq_d, i, [128, KO, 512]) for i in range(2)]
                wk_sb = [wpiece(wk_d, i, [128, KO, 512]) for i in range(2)]
                wv_sb = [wpiece(wv_d, i, [128, KO, 512]) for i in range(2)]

                # ---- P1: LN1 over all 1024 tokens ----
                ps_s1 = [psum() for _ in range(2)]
                ps_s2 = [psum() for _ in range(2)]
                for ko in range(KO):
                    sqk = sb.tile([128, S], bf, tag="kslab", bufs=4, name="sqk")
                    nc.scalar.square(sqk[:], xb8[ko][:])
                    for tb in range(2):
                        nc.tensor.matmul(ps_s1[tb][:], ones_b[:], xb8[ko][:, ts(tb, 512)],
                                         start=(ko == 0), stop=(ko == KO - 1))
                        nc.tensor.matmul(ps_s2[tb][:], ones_b[:], sqk[:, ts(tb, 512)],
                                         start=(ko == 0), stop=(ko == KO - 1))

                mu01 = sb.tile([128, 1024], bf, tag="stats16", bufs=4, name="mu01")
                rstd01 = sb.tile([128, 1024], bf, tag="stats16", bufs=4, name="rstd01")
                for tb in range(2):
                    mu = tmpf()
                    nc.vector.tensor_scalar_mul(mu[:], ps_s1[tb][:], 1.0 / D)
                    ex2 = tmpf()
                    nc.vector.tensor_scalar_mul(ex2[:], ps_s2[tb][:], 1.0 / D)
                    var = tmpf()
                    nc.vector.tensor_tensor(var[:], mu[:], mu[:], ALU.mult)
                    nc.vector.tensor_tensor(var[:], ex2[:], var[:], ALU.subtract)
                    sd = tmpf()
                    nc.scalar.activation(sd[:], var[:], AF.Sqrt, bias=eps_ap[:])
                    nc.vector.tensor_copy(mu01[:, ts(tb, 512)], mu[:])
                    with nc.allow_low_precision(reason="bf16 LN rstd"):
                        nc.vector.reciprocal(rstd01[:, ts(tb, 512)], sd[:])

                g16 = []
                for ko in range(KO):
                    g = sb.tile([128, S], bf, tag="act2k", bufs=16, name="g16")
                    tm = sb.tile([128, S], bf, tag="kslab", bufs=4, name="tm")
                    nc.vector.tensor_tensor(tm[:], xb8[ko][:], mu01[:], ALU.subtract)
                    nc.vector.tensor_tensor(g[:], tm[:], rstd01[:], ALU.mult)
                    g16.append(g)

                # ---- P2: projections + rope (swap via PE perm matmul) ----
                qr8 = []
                for jo in range(KO):
                    pq = psum()
                    for ko in range(KO):
                        nc.tensor.matmul(pq[:], wq_sb[jo // 4][:, ko, ts(jo % 4, 128)],
                                         g16[ko][:, 0:Q], start=(ko == 0), stop=(ko == KO - 1))
                    qa = sb.tile([128, 512], bf, tag="qslab", bufs=6, name="qa")
                    nc.scalar.add(qa[:, :Q], pq[:, :Q], bq_s[:, jo:jo + 1])
                    psw = psum()
                    nc.tensor.matmul(psw[:, :Q], pswap, qa[:, :Q], start=True, stop=True)
                    t1 = sb.tile([128, 512], bf, tag="qslab", bufs=6, name="qt1")
                    nc.vector.tensor_tensor(t1[:, :Q], qa[:, :Q], csb[:, 0:Q], ALU.mult)
                    t2 = sb.tile([128, 512], bf, tag="qslab", bufs=6, name="qt2")
                    nc.vector.tensor_tensor(t2[:, :Q], psw[:, :Q], ssb[:, 0:Q], ALU.mult)
                    qr = sb.tile([128, Q], bf, tag="act1k", bufs=16, name="qr")
                    nc.vector.tensor_tensor(qr[:], t1[:, :Q], t2[:, :Q], ALU.add)
                    qr8.append(qr)

                kr8 = []
                for jo in range(KO):
                    ka = sb.tile([128, S], bf, tag="kslab", bufs=4, name="ka")
                    kr = sb.tile([128, S], bf, tag="act2k", bufs=16, name="kr")
                    psw2 = psum2()
                    for tb in range(2):
                        pk = psum()
                        for ko in range(KO):
                            nc.tensor.matmul(pk[:], wk_sb[jo // 4][:, ko, ts(jo % 4, 128)],
                                             g16[ko][:, ts(tb, 512)],
                                             start=(ko == 0), stop=(ko == KO - 1))
                        nc.scalar.add(ka[:, ts(tb, 512)], pk[:], bk_s[:, jo:jo + 1])
                        nc.tensor.matmul(psw2[:, ts(tb, 512)], pswap, ka[:, ts(tb, 512)],
                                         start=True, stop=True)
                    t1 = sb.tile([128, S], bf, tag="kslab", bufs=4, name="kt1")
                    nc.vector.tensor_tensor(t1[:], ka[:], csb[:], ALU.mult)
                    t2 = sb.tile([128, S], bf, tag="kslab", bufs=4, name="kt2")
                    nc.vector.tensor_tensor(t2[:], psw2[:], ssb[:], ALU.mult)
                    nc.vector.tensor_tensor(kr[:], t1[:], t2[:], ALU.add)
                    kr8.append(kr)

                # v, token-major, with a ones-column per head (denominator trick)
                v_sb = sb.tile([128, KO, H, 66], bf, tag="m16v", bufs=1, name="vsb")
                nc.vector.memset(v_sb[:, :, :, 64:65], 1.0)
                for to in range(KO):
                    for nb in range(2):
                        pv = psum()
                        for ko in range(KO):
                            nc.tensor.matmul(pv[:], g16[ko][:, ts(to, 128)],
                                             wv_sb[nb][:, ko, :],
                                             start=(ko == 0), stop=(ko == KO - 1))
                        nc.scalar.copy(v_sb[:, to, nb * 8:(nb + 1) * 8, 0:64],
                                       pv[:].rearrange("p (h d) -> p h d", d=64))

                # ---- P3: attention (scoresT layout), head pairs interleaved ----
                wo_sb = [wpiece(wo_d, i, [128, KO, 512]) for i in range(2)]
                oT8 = [sb.tile([128, Q], bf, tag="act1k", bufs=16, name="oT")
                       for _ in range(KO)]
                for hp in range(8):
                    jo = hp
                    probs = {0: [], 1: []}
                    for half in range(4):          # 2 key-tiles per chunk
                        pbig = {}
                        for sub in range(2):       # the 2 heads of the pair
                            r0 = sub * 64
                            big = psum2()
                            for kk in range(2):
                                kt = half * 2 + kk
                                nc.tensor.matmul(big[:, ts(kk, 512)],
                                                 kr8[jo][r0:r0 + 64, ts(kt, 128)],
                                                 qr8[jo][r0:r0 + 64, :],
                                                 start=True, stop=True,
                                                 tile_position=(r0, 0))
                            pbig[sub] = big
                        for sub in range(2):
                            pb = sb.tile([128, 1024], bf, tag="xb8", bufs=8, name="pb")
                            nc.scalar.activation(pb[:], pbig[sub][:], AF.Exp, scale=0.125)
                            probs[sub].append(pb)
                    po2 = {}
                    for sub in range(2):
                        h = 2 * hp + sub
                        po = psum()
                        for kt in range(KO):
                            nc.tensor.matmul(po[0:65, :], v_sb[:, kt, h, 0:65],
                                             probs[sub][kt // 2][:, ts(kt % 2, 512)],
                                             start=(kt == 0), stop=(kt == KO - 1))
                        po2[sub] = po
                    prbp2 = psum2()
                    prbp = prbp2[:, 0:512]
                    # sub1: copy unnormalized o + denom to SBUF, shift rows to
                    # 64:128 via identity matmul into po's free rows
                    po1 = po2[1]
                    o65 = sb.tile([65, 512], bf, tag="o16", bufs=2, name="o65")
                    nc.vector.tensor_copy(o65[:], po1[0:65, :])
                    nc.tensor.matmul(po1[64:128, :], id64, o65[0:64, :],
                                     start=True, stop=True)
                    rcp = sb.tile([65, 512], bf, tag="rcp", bufs=2, name="rcp")
                    with nc.allow_low_precision(reason="bf16 softmax denominator"):
                        nc.vector.reciprocal(rcp[64:65, :], po2[0][64:65, :])
                        nc.vector.reciprocal(o65[64:65, :], o65[64:65, :])
                    nc.tensor.matmul(prbp[0:64, :], ones_b[64:65, 0:64], rcp[64:65, :],
                                     start=True, stop=True, tile_position=(64, 0))
                    nc.tensor.matmul(prbp[64:128, :], ones_b[64:65, 0:64], o65[64:65, :],
                                     start=True, stop=True, tile_position=(64, 64))
                    rb = sb.tile([128, 512], bf, tag="rb", bufs=2, name="rb")
                    nc.vector.tensor_copy(rb[:], prbp[:])
                    nc.vector.tensor_tensor(oT8[jo][0:64, :], po2[0][0:64, :],
                                            rb[0:64, :], ALU.mult)
                    nc.vector.tensor_tensor(oT8[jo][64:128, :], po1[64:128, :],
                                            rb[64:128, :], ALU.mult)

                # ---- P4: attn out + gated residual ----
                x2 = []
                for do in range(KO):
                    py = psum()
                    for ko in range(KO):
                        nc.tensor.matmul(py[:], wo_sb[do // 4][:, ko, ts(do % 4, 128)],
                                         oT8[ko][:], start=(ko == 0), stop=(ko == KO - 1))
                    t = tmpf()
                    nc.scalar.activation(t[:], py[:], AF.Identity,
                                         bias=bo_s[:, do:do + 1],
                                         scale=gm_s[:, do:do + 1])
                    xx = sb.tile([128, Q], f32, tag="act2k", bufs=16, name="x2")
                    nc.vector.tensor_tensor(xx[:], t[:], xskip[:, do], ALU.add)
                    x2.append(xx)

                # ---- P5: LN2 (512 tokens) ----
                p1 = psum()
                p2 = psum()
                x2b = []
                for ko in range(KO):
                    xc = sb.tile([128, Q], bf, tag="act1k", bufs=16, name="x2b")
                    nc.scalar.copy(xc[:], x2[ko][:])
                    x2b.append(xc)
                    sq2 = sb.tile([128, S], bf, tag="kslab", bufs=4, name="sq2")
                    nc.scalar.square(sq2[:, 0:Q], x2[ko][:])
                    nc.tensor.matmul(p1[:], ones_b[:], xc[:], start=(ko == 0),
                                     stop=(ko == KO - 1))
                    nc.tensor.matmul(p2[:], ones_b[:], sq2[:, 0:Q], start=(ko == 0),
                                     stop=(ko == KO - 1))
                mu = tmpf()
                nc.vector.tensor_scalar_mul(mu[:], p1[:], 1.0 / D)
                ex2 = tmpf()
                nc.vector.tensor_scalar_mul(ex2[:], p2[:], 1.0 / D)
                var = tmpf()
                nc.vector.tensor_tensor(var[:], mu[:], mu[:], ALU.mult)
                nc.vector.tensor_tensor(var[:], ex2[:], var[:], ALU.subtract)
                sd = tmpf()
                nc.scalar.activation(sd[:], var[:], AF.Sqrt, bias=eps_ap[:])
                mu16 = sb.tile([128, 512], bf, tag="stats16", bufs=4, name="mu16b")
                nc.vector.tensor_copy(mu16[:], mu[:])
                rstd16 = sb.tile([128, 512], bf, tag="stats16", bufs=4, name="rstd16b")
                with nc.allow_low_precision(reason="bf16 LN rstd"):
                    nc.vector.reciprocal(rstd16[:], sd[:])
                g2 = []
                for ko in range(KO):
                    tm = sb.tile([128, 512], bf, tag="qslab", bufs=6, name="tm2")
                    nc.vector.tensor_tensor(tm[:], x2b[ko][:], mu16[:], ALU.subtract)
                    gk = sb.tile([128, Q], bf, tag="act1k", bufs=16, name="g2")
                    nc.vector.tensor_tensor(gk[:], tm[:], rstd16[:], ALU.mult)
                    g2.append(gk)

                # ---- P6/P7: MLP ----
                w1_sb = [wpiece(w1_d, i, [128, KO, 512]) for i in range(8)]
                w2_sb = [wpiece(w2_d, i, [128, 32, 128]) for i in range(8)]
                m16 = sb.tile([128, 32, Q], bf, tag="m16v", bufs=1, name="m16")
                for mo in range(32):
                    pm = psum()
                    for ko in range(KO):
                        nc.tensor.matmul(pm[:], w1_sb[mo // 4][:, ko, ts(mo % 4, 128)],
                                         g2[ko][:], start=(ko == 0),
                                         stop=(ko == KO - 1))
                    nc.scalar.activation(m16[:, mo], pm[:], AF.Gelu_apprx_tanh,
                                         bias=b1_s[:, mo:mo + 1], scale=1.0)
                yt_r = yt_d.rearrange("(ko p) t -> p ko t", p=128)
                for do in range(KO):
                    pz = psum()
                    for ko in range(32):
                        nc.tensor.matmul(pz[:], w2_sb[do][:, ko, :],
                                         m16[:, ko, :], start=(ko == 0), stop=(ko == 31))
                    t = tmpf()
                    nc.scalar.activation(t[:], pz[:], AF.Identity,
                                         bias=b2_s[:, do:do + 1],
                                         scale=gp_s[:, do:do + 1])
                    nc.vector.tensor_tensor(x2[do][:], t[:], x2[do][:], ALU.add)
                    nc.gpsimd.dma_start(yt_r[:, do, :], x2[do][:])

    nc.compile()
    return nc


# ----------------------------------------------------------------------------
# host wrapper
# ----------------------------------------------------------------------------

def _prep_shared(inputs):
    x = np.asarray(inputs["x"], np.float32)
    c = np.asarray(inputs["c"], np.float32)
    w_ada = np.asarray(inputs["w_ada"], np.float32)
    b_ada = np.asarray(inputs["b_ada"], np.float32)
    w_qkv = np.asarray(inputs["w_qkv"], np.float32)
    w_ao = np.asarray(inputs["w_attn_out"], np.float32)
    w_m1 = np.asarray(inputs["w_mlp1"], np.float32)
    w_m2 = np.asarray(inputs["w_mlp2"], np.float32)

    mod = c @ w_ada + b_ada
    sh_msa, sc_msa, g_msa, sh_mlp, sc_mlp, g_mlp = np.split(mod, 6, axis=1)
    ln1 = np.asarray(inputs["w_ln1"], np.float32) * (1.0 + sc_msa)   # [4, D]
    ln2 = np.asarray(inputs["w_ln2"], np.float32) * (1.0 + sc_mlp)

    shared = {}
    for b in range(B):
        Wq = w_qkv[:, :D] * ln1[b][:, None]
        Wk = w_qkv[:, D:2 * D] * ln1[b][:, None]
        Wv = w_qkv[:, 2 * D:] * ln1[b][:, None]
        bqkv = sh_msa[b] @ w_qkv
        W1 = w_m1 * ln2[b][:, None]
        bm1 = sh_mlp[b] @ w_m1 + np.asarray(inputs["b_mlp1"], np.float32)
        bias = np.concatenate([
            _pvec(bqkv[:D]), _pvec(bqkv[D:2 * D]),
            _pvec((bqkv[2 * D:] @ w_ao) * g_msa[b]),
            _pvec(g_msa[b]), _pvec(bm1),
            _pvec(np.asarray(inputs["b_mlp2"], np.float32) * g_mlp[b]),
            _pvec(g_mlp[b]),
        ], axis=1)
        shared[b] = dict(
            wq=_pieces(Wq, 512), wk=_pieces(Wk, 512), wv=_pieces(Wv, 512),
            wm1=_pieces(W1, 512),
            bias=np.ascontiguousarray(bias),
        )
    wao_p = _pieces(w_ao, 512)
    wm2_p = _pieces(w_m2, 128)
    cos = np.asarray(inputs["cos"], np.float32)
    sin = np.asarray(inputs["sin"], np.float32)
    return shared, wao_p, wm2_p, x, cos, sin


def _make_in_maps(inputs):
    shared, wao_p, wm2_p, x, cos, sin = _prep_shared(inputs)
    pm = _perm_mats()
    in_maps = []
    for core in range(8):
        b, half = core // 2, core % 2
        qlo = half * Q
        order = np.concatenate([np.arange(qlo, qlo + Q), np.arange(0, qlo),
                                np.arange(qlo + Q, S)])
        xT = x[b][order].T
        cosT = cos[order].T                      # [32, S]
        sinT = sin[order].T
        cc = np.concatenate([cosT] * 4, 0).astype(BF)
        ss = np.concatenate([-sinT, sinT, -sinT, sinT], 0).astype(BF)
        cs = np.concatenate([cc, ss], 1)         # [128, 2S]
        sh = shared[b]
        in_maps.append({
            "xb": np.ascontiguousarray(xT.astype(BF)),
            "xskip": np.ascontiguousarray(xT[:, :Q].astype(np.float32)),
            "wq": sh["wq"], "wk": sh["wk"], "wv": sh["wv"],
            "wao": wao_p, "wm1": sh["wm1"], "wm2": wm2_p,
            "cs": np.ascontiguousarray(cs),
            "bias": sh["bias"],
            "pm": pm,
        })
    return in_maps


def kernel(**inputs):
    from concourse import bass_utils

    if "nc" not in _CACHE:
        _CACHE["nc"] = _build_program()
    nc = _CACHE["nc"]

    in_maps = _make_in_maps(inputs)
    res = bass_utils.run_bass_kernel_spmd(nc, in_maps, core_ids=list(range(8)))

    y = np.zeros((B, S, D), np.float32)
    for core in range(8):
        b, half = core // 2, core % 2
        qlo = half * Q
        y[b, qlo:qlo + Q] = res.results[core]["yt"].T
    return y
